# revision 23
# baseline (speedup 1.0000x reference)
"""GCN layer kernel for 8 trn2 NeuronCores.

Math:  out = D (A + I) D feature W^T + b      (D = diag(hat_d))
Rewritten with g = (hat_d * feature) @ W^T  (the linear commutes with the
row-scaling and the SpMM):
    out = hat_d * (A @ g) + hat_d * g + b

Sharding: A row-sharded across 8 cores (2048 rows each). Each core
computes full g locally from a replicated feature^T: an 8MB AllGather
was measured at ~110us on this fabric — far more than the ~55us of
replicated phase-1 PE work it would save, so no collectives.

Device layout: the big matmul is computed transposed,
out_sh^T[o, m] = sum_j g[j, o] * A_sh^T[j, m], so g tiles are the
stationary operand and the A shard (pre-transposed on the host — lhsT
layout prep) is the moving operand. The host applies an "own rows
first" node permutation to the j axis of A^T / feature^T / hat_d so
the same SPMD program works on every core. Each (k, h) weight load
covers four 512-col matmuls so LDWEIGHTS stays hidden; the main loop
streams at the measured 216ns/512-col PE roofline.

feature streams as fp8 e3m4 (pre-scaled x2 on host, clipped to range,
with the exact 1/2 folded into the fp16 W^T): phase-1 was measured
DMA-bound (16.8MB of fp16 features ~ 67us at the ~250GB/s per-core cap
vs ~62us of PE work), so halving the feature bytes makes phase-1
PE-bound and also halves its LDWEIGHTS stream (FWL reads 4 fp8/word).
The zero-mean quantization error needs no compensation — measured
same-epoch A/B win of ~5-8us over fp16 features, end-to-end relative
error 1.38e-2 (budget 2e-2; emulation->HW fidelity ~1e-5). A stays
fp16: fp8-A requires a mean-subtraction correction whose DVE colsum
folds overflow phase-1's DVE budget and cost ~15us net (measured).
Accumulation and epilogue are fp32; the output is written fp16 (host
upcasts) to shave the serial tail DMA.

Rejected on measurement: collectives (above), m-chunked epilogue
overlap (doubles LDWEIGHTS, +17us), fp8 A (+15us via DVE overflow),
DoubleRow fp8 g (2.8e-2 rel, fails the gate). Run-to-run variance is
dominated by chip power state (P0 2.0GHz / HAM 13/16 epochs).
Note: the gsum/gsum16/corr_sb tiles below are unused remnants of the
fp8-A experiment kept to preserve the exact validated SBUF layout;
they emit no instructions.
"""

import os

import ml_dtypes
import numpy as np

import concourse.mybir as mybir
import concourse.tile as tile
from concourse import bacc
from concourse.bass_utils import run_bass_kernel_spmd
from concourse.masks import make_identity

N = 16384
F = 512
O = 256
NCORES = 8
SH = N // NCORES
JT = N // 128
MT = SH // 128
NB = 2048

F32 = mybir.dt.float32
F16 = mybir.dt.float16
F8 = mybir.dt.float8e3  # e3m4

_CACHE = {}


def build_program():
    nc = bacc.Bacc("TRN2", target_bir_lowering=False, debug=False,
                   num_devices=NCORES, dynamic_dma_scratch_size=8192)

    at = nc.dram_tensor("at", [N, SH], F16, kind="ExternalInput").ap()
    ft = nc.dram_tensor("ft", [F, N], F8, kind="ExternalInput").ap()
    hdt = nc.dram_tensor("hdt", [128, JT], F32, kind="ExternalInput").ap()
    hdo = nc.dram_tensor("hdo", [1, SH], F16, kind="ExternalInput").ap()
    wt = nc.dram_tensor("wt", [F, O], F16, kind="ExternalInput").ap()
    bvec = nc.dram_tensor("bvec", [O, 1], F32, kind="ExternalInput").ap()
    outT = nc.dram_tensor("outT", [O, SH], F16, kind="ExternalOutput").ap()

    add = mybir.AluOpType.add
    mult = mybir.AluOpType.mult

    with tile.TileContext(nc) as tc:
        with (
            tc.tile_pool(name="const", bufs=1) as constp,
            tc.tile_pool(name="gpool", bufs=1) as gp,
            tc.tile_pool(name="fslab", bufs=12) as fsp,
            tc.tile_pool(name="aslab", bufs=12) as asp,
            tc.tile_pool(name="tout", bufs=4) as wp,
            tc.tile_pool(name="scr", bufs=2) as scp,
        ):
            qs = [nc.sync, nc.scalar]

            half_slabs = [[], []]
            for hb in range(2):
                for fc in range(4):
                    s = fsp.tile([128, NB // 2], F8, tag="fs",
                                 name=f"fs0{hb}_{fc}")
                    qs[fc % 2].dma_start(
                        out=s[:],
                        in_=ft[fc * 128:(fc + 1) * 128,
                               hb * (NB // 2):(hb + 1) * (NB // 2)])
                    half_slabs[hb].append(s)

            ident = constp.tile([128, 128], F32, tag="ident")
            make_identity(nc, ident[:])

            wt_sb = constp.tile([128, 4 * O], F16, tag="wt")
            for fc in range(4):
                nc.scalar.dma_start(out=wt_sb[:, fc * O:(fc + 1) * O],
                                    in_=wt[fc * 128:(fc + 1) * 128, :])
            hdt_sb = constp.tile([128, JT], F32, tag="hdt")
            nc.scalar.dma_start(out=hdt_sb[:], in_=hdt[:, :])

            g_sb = gp.tile([128, JT * O], F16, tag="g")
            e_sb = gp.tile([128, 2 * SH], F32, tag="e")
            # colsum accumulator (fp32), folded 4096 -> 256 at the end
            gsum = gp.tile([128, 16 * O], F32, tag="gsum")
            gsum16 = gp.tile([128, O], F16, tag="gsum16")
            # corr[o] = 0.5 * sum_j g[j, o], o-half h in column h
            corr_sb = constp.tile([128, 2], F32, tag="corr")

            with tc.tile_pool(name="ps1", bufs=2, space="PSUM") as ps1:
                for jb in range(N // NB):
                    if jb == 0:
                        slabs = None
                    else:
                        slabs = []
                        for fc in range(4):
                            s = fsp.tile([128, NB], F8, tag="fs",
                                         name=f"fs{jb}_{fc}")
                            qs[fc % 2].dma_start(
                                out=s[:],
                                in_=ft[fc * 128:(fc + 1) * 128,
                                       jb * NB:(jb + 1) * NB])
                            slabs.append(s)
                    for jj in range(NB // 128):
                        j = jb * (NB // 128) + jj
                        if jb == 0:
                            sl_group = half_slabs[jj // 8]
                            col = (jj % 8) * 128
                        else:
                            sl_group = slabs
                            col = jj * 128
                        pfw = ps1.tile([128, O], F32, tag="fw", bufs=6)
                        for fc in range(4):
                            nc.tensor.matmul(
                                pfw[:],
                                lhsT=sl_group[fc][:, col:col + 128],
                                rhs=wt_sb[:, fc * O:(fc + 1) * O],
                                start=(fc == 0), stop=(fc == 3))
                        if j % 2 == 0:
                            nc.vector.tensor_scalar_mul(
                                g_sb[:, j * O:(j + 1) * O], pfw[:],
                                hdt_sb[:, j:j + 1])
                        else:
                            nc.scalar.mul(
                                g_sb[:, j * O:(j + 1) * O], pfw[:],
                                hdt_sb[:, j:j + 1])
                    if jb == 0:
                        for jj in range(MT):
                            for h in range(2):
                                sc = scp.tile([128, 128], F32, tag="sc")
                                nc.vector.tensor_scalar_mul(
                                    sc[:],
                                    g_sb[:, jj * O + h * 128:
                                         jj * O + (h + 1) * 128],
                                    hdt_sb[:, jj:jj + 1])
                                ptp = ps1.tile([128, 128], F32, tag="tp",
                                               bufs=2)
                                nc.tensor.transpose(ptp[:], sc[:], ident[:])
                                nc.vector.tensor_copy(
                                    e_sb[:, h * SH + jj * 128:
                                         h * SH + (jj + 1) * 128],
                                    ptp[:])

            b_sb = constp.tile([128, 2], F32, tag="b")
            for h in range(2):
                nc.scalar.dma_start(out=b_sb[:, h:h + 1],
                                    in_=bvec[h * 128:(h + 1) * 128, :])
            hd_bc = constp.tile([128, SH], F16, tag="hdbc")
            nc.scalar.dma_start(out=hd_bc[:],
                                in_=hdo[0:1, :].to_broadcast((128, SH)))

            with tc.tile_pool(name="ps2", bufs=1, space="PSUM") as psp:
                accs = [psp.tile([128, SH], F32, tag=f"acc{h}", name=f"acc{h}")
                        for h in range(2)]
                for k in range(JT):
                    sl = asp.tile([128, SH], F16, tag="as")
                    qs[k % 2].dma_start(out=sl[:],
                                        in_=at[k * 128:(k + 1) * 128, :])
                    for h in range(2):
                        lhs = g_sb[:, k * O + h * 128:k * O + (h + 1) * 128]
                        for mc in range(4):
                            nc.tensor.matmul(
                                accs[h][:, mc * 512:(mc + 1) * 512],
                                lhsT=lhs,
                                rhs=sl[:, mc * 512:(mc + 1) * 512],
                                start=(k == 0), stop=(k == JT - 1))

                for h in range(2):
                    for c in range(4):
                        cs = slice(c * 512, (c + 1) * 512)
                        t = wp.tile([128, 512], F32, tag="t")
                        nc.vector.tensor_tensor(t[:], accs[h][:, cs],
                                                hd_bc[:, cs], mult)
                        t16 = wp.tile([128, 512], F16, tag="t16")
                        nc.vector.scalar_tensor_tensor(
                            t16[:], in0=t[:], scalar=b_sb[:, h:h + 1],
                            in1=e_sb[:, h * SH + c * 512:
                                     h * SH + (c + 1) * 512],
                            op0=add, op1=add)
                        qs[(h + c) % 2].dma_start(
                            out=outT[h * 128:(h + 1) * 128, cs], in_=t16[:])

    nc.compile()
    return nc


def prep_inputs(A, hat_d, feature, W, b):
    A = np.ascontiguousarray(np.asarray(A, dtype=np.float32))
    hat_d = np.ascontiguousarray(np.asarray(hat_d, dtype=np.float32))
    feature = np.ascontiguousarray(np.asarray(feature, dtype=np.float32))
    W = np.asarray(W, dtype=np.float32)
    b = np.asarray(b, dtype=np.float32)

    # features are streamed fp8 e3m4, pre-scaled x2 (clipped to e3m4 range)
    # with the exact 1/2 folded into the fp16 W^T; zero-mean quantization
    # error needs no compensation term. Measured end-to-end rel ~1.4e-2.
    featT = np.ascontiguousarray(
        np.clip(feature.T * np.float32(2.0), -15.0, 15.0)
        .astype(ml_dtypes.float8_e3m4))
    wt = np.ascontiguousarray((W.T * np.float32(0.5)).astype(np.float16))
    b2 = np.ascontiguousarray(b.reshape(O, 1))

    in_maps = []
    for c in range(NCORES):
        r0, r1 = c * SH, (c + 1) * SH
        rows = A[r0:r1].astype(np.float16)
        at_c = np.empty((N, SH), dtype=np.float16)
        at_c[:SH] = rows[:, r0:r1].T
        at_c[SH:SH + r0] = rows[:, :r0].T
        at_c[SH + r0:] = rows[:, r1:].T

        ft_c = np.empty((F, N), dtype=ml_dtypes.float8_e3m4)
        ft_c[:, :SH] = featT[:, r0:r1]
        ft_c[:, SH:SH + r0] = featT[:, :r0]
        ft_c[:, SH + r0:] = featT[:, r1:]

        hd_c = np.concatenate([hat_d[r0:r1], hat_d[:r0], hat_d[r1:]])
        hdt_c = np.ascontiguousarray(hd_c.reshape(JT, 128).T)
        hdo_c = np.ascontiguousarray(
            hat_d[r0:r1].reshape(1, SH).astype(np.float16))

        in_maps.append({
            "at": at_c,
            "ft": ft_c,
            "hdt": hdt_c,
            "hdo": hdo_c,
            "wt": wt,
            "bvec": b2,
        })
    return in_maps


last_exec_time_ns = None
last_results = None


def kernel(A, hat_d, feature, W, b):
    global last_exec_time_ns, last_results
    if "nc" not in _CACHE:
        _CACHE["nc"] = build_program()
    nc = _CACHE["nc"]

    in_maps = prep_inputs(A, hat_d, feature, W, b)
    trace = bool(int(os.environ.get("KERNEL_TRACE", "0")))
    res = run_bass_kernel_spmd(nc, in_maps, list(range(NCORES)), trace=trace)
    last_exec_time_ns = res.exec_time_ns
    last_results = res

    out = np.empty((N, O), dtype=np.float32)
    for c in range(NCORES):
        out[c * SH:(c + 1) * SH] = res.results[c]["outT"].T.astype(np.float32)
    return out
